# revision 59
# baseline (speedup 1.0000x reference)
"""Trainium2 Bass kernel for nn_MinimalAttention (B=1, S=4096, d_model=768,
H=12, Dh=64, post-softmax causal mask).

Sharding (8 cores): 4 head-groups (3 heads each) x 2 sequence shards.
Each seq shard owns 2048 query rows as 16 128-row subtiles, mod-4
interleaved across the sequence so the causal-mask work pattern is
identical on every core (the SPMD program is branch-free; all per-core
variation is input data: weight slices, pre-gathered xTq columns, mask
tiles).

Per core:
  K^T/Q^T projections in bf16 (PSUM f32, bias on DVE), V projection in
  bf16; scores^T = K_h^T slice x Q_h^T per 128-key tile (bf16, PSUM
  f32).

  exp is split across two engines by causality: key tiles at or before
  the diagonal (needed by the numerator) get exact exp on ScalarE;
  strictly-future key tiles (denominator-only) get a Schraudolph
  fast-exp on the Vector engine: one tensor_scalar (x*A + B) written as
  int16 and bitcast to bf16 approximates exp(x) to ~+-3% with ~zero
  mean error, which averages out inside the 4096-key softmax
  denominator.

  y accumulation: tiles causal on both shards use E @ [V|ones] (col 64
  accumulates the denominator); the 2 maybe-masked diagonal tiles per
  128-row block use a GpSimd-masked em (numerator, 64 cols) plus full-E
  x ones-column matmuls (denominator, 1 col) that also cover all future
  tiles. Per-unit batched reciprocal + broadcast scale, PE transpose to
  y^T, DVE copies y^T out of PSUM; partial output projection
  outT = W_out_slice^T chunk @ y^T.

  Scheduling: units are processed in pairs (qb=0 with 3, 1 with 2) with
  their 2-key-tile score groups interleaved one-by-one through three
  rotating 2-bank PSUM pools, so the exact-exp (ScalarE-heavy late-qb)
  and fast-exp (DVE-heavy early-qb) streams stay concurrently busy. The
  first pair's groups are woven between the 8 K-projection slabs; score
  production gets a priority boost (PRIO_OFF) over numerator drain.

Host sums the 4 head-group partials per shard, adds b_out, and scatters
the interleaved rows back.
"""
import sys

sys.path.insert(0, "/opt/trn_rl_repo")

import numpy as np
import ml_dtypes

S, D, H, DH = 4096, 768, 12, 64
N_CORES = 8
GD = 192          # head-group dims (3 heads)
LT = 16           # local 128-row subtiles per core (2048 q rows)
NK = 32           # key tiles

# Schraudolph fast-exp constants for bf16-as-int16 output:
# bf16_bits(exp(x)) ~= round(x * 128/ln2 + (127*128 - C)); C tuned so the
# mean multiplicative error is ~0 (the elementwise error is +-3%).
SCH_A = 128.0 / np.log(2.0)
SCH_B = 127.0 * 128.0 - 7.35
PRIO_OFF = 300

_cache = {}


def _g_of(s, t):
    k = t // 2
    if s == 0:
        return 4 * k + (0 if t % 2 == 0 else 3)
    return 4 * k + (1 if t % 2 == 0 else 2)


def _qcols(s):
    idx = []
    for t in range(LT):
        g = _g_of(s, t)
        idx.extend(range(g * 128, g * 128 + 128))
    return np.array(idx)


def _masks(s):
    """Masks for the 2 maybe-masked key tiles per tile parity.

    For tile parity `par`, the maybe-masked key tiles are j = 2*par and
    2*par + 1 within the 4-tile diagonal window; delta(par, s) = _g_of(s,
    par) mod 4 selects which is the triangular one (earlier: all-ones,
    later: all-zero).
    """
    tri = (np.arange(128)[:, None] <= np.arange(128)[None, :]).astype(np.float32)
    M = np.zeros((2, 2, 128, 128), np.float32)
    for par in (0, 1):
        delta = _g_of(s, par)
        for jp in range(2):
            j = 2 * par + jp
            if j < delta:
                M[par, jp] = 1.0
            elif j == delta:
                M[par, jp] = tri
    return M


def _build_program(reps=1):
    import concourse.bass as bass
    import concourse.mybir as mybir
    import concourse.tile as tile
    from concourse import bacc

    f32 = mybir.dt.float32
    bf16 = mybir.dt.bfloat16
    i16 = mybir.dt.int16
    Exp = mybir.ActivationFunctionType.Exp
    mult = mybir.AluOpType.mult
    add = mybir.AluOpType.add

    nc = bacc.Bacc(
        "TRN2",
        target_bir_lowering=False,
        debug=False,
        enable_asserts=False,
        num_devices=N_CORES,
    )

    d_xT = nc.dram_tensor("xt_in", [D, S], bf16, kind="ExternalInput").ap()
    d_xTq = nc.dram_tensor("xtq_in", [D, 2048], bf16, kind="ExternalInput").ap()
    d_wq = nc.dram_tensor("wq_in", [D, GD], bf16, kind="ExternalInput").ap()
    d_wk = nc.dram_tensor("wk_in", [D, GD], bf16, kind="ExternalInput").ap()
    d_wv = nc.dram_tensor("wv_in", [D, GD], bf16, kind="ExternalInput").ap()
    d_bq = nc.dram_tensor("bq_in", [GD, 1], f32, kind="ExternalInput").ap()
    d_bk = nc.dram_tensor("bk_in", [GD, 1], f32, kind="ExternalInput").ap()
    d_bvb = nc.dram_tensor("bvb_in", [128, GD], bf16, kind="ExternalInput").ap()
    d_wo = nc.dram_tensor("wo_in", [GD, D], bf16, kind="ExternalInput").ap()
    d_mm = nc.dram_tensor("mm_in", [2, 2, 128, 128], bf16, kind="ExternalInput").ap()
    d_id = nc.dram_tensor("id_in", [128, 128], bf16, kind="ExternalInput").ap()
    d_out = nc.dram_tensor("outt_out", [D, 2048], f32, kind="ExternalOutput").ap()

    with tile.TileContext(nc) as tc:
        xTr = d_xT.rearrange("(k p) n -> p k n", p=128)     # [128, 6, 4096]
        xTqr = d_xTq.rearrange("(k p) n -> p k n", p=128)   # [128, 6, 2048]
        wqr = d_wq.rearrange("(k p) m -> p k m", p=128)
        wkr = d_wk.rearrange("(k p) m -> p k m", p=128)
        wvr = d_wv.rearrange("(k p) m -> p k m", p=128)
        mmr = d_mm.rearrange("a b p n -> p a b n")
        with tc.tile_pool(name="const", bufs=1) as cp:
            wq_sb = cp.tile([128, 6, GD], bf16, tag="wq")
            wk_sb = cp.tile([128, 6, GD], bf16, tag="wk")
            wv_sb = cp.tile([128, 6, GD], bf16, tag="wv")
            wo0 = cp.tile([128, D], bf16, tag="wo0")
            wo1 = cp.tile([64, D], bf16, tag="wo1")
            bq0 = cp.tile([128, 1], f32, tag="bq0")
            bq1 = cp.tile([64, 1], f32, tag="bq1")
            bk0 = cp.tile([128, 1], f32, tag="bk0")
            bk1 = cp.tile([64, 1], f32, tag="bk1")
            bvb = cp.tile([128, GD], bf16, tag="bvb")
            mm_sb = cp.tile([128, 2, 2, 128], bf16, tag="mm")
            id_sb = cp.tile([128, 128], bf16, tag="ident")
            ones = cp.tile([128, 1], bf16, tag="ones")
            zvb = cp.tile([128, 65], bf16, tag="zvb")
            KT0 = cp.tile([128, S], bf16, tag="KT0")
            KT1 = cp.tile([64, S], bf16, tag="KT1")
            QT0 = cp.tile([128, 2048], bf16, tag="QT0")
            QT1 = cp.tile([64, 2048], bf16, tag="QT1")
            Vb = cp.tile([128, 3, NK, 65], bf16, tag="Vb")
            yT0 = cp.tile([128, 2048], bf16, tag="yT0")
            yT1 = cp.tile([64, 2048], bf16, tag="yT1")

            nc.sync.dma_start(out=wq_sb[:], in_=wqr[:])
            nc.sync.dma_start(out=wk_sb[:], in_=wkr[:])
            nc.sync.dma_start(out=bq0[:], in_=d_bq[0:128, :])
            nc.sync.dma_start(out=bq1[:], in_=d_bq[128:GD, :])
            nc.sync.dma_start(out=bk0[:], in_=d_bk[0:128, :])
            nc.sync.dma_start(out=bk1[:], in_=d_bk[128:GD, :])
            nc.vector.memset(ones[:], 1.0)
            nc.vector.memset(zvb[:], 0.0)
            nc.vector.memset(Vb[:, :, :, 64:65], 1.0)

            import contextlib
            loop_ctx = tc.For_i(0, reps, 1) if reps > 1 else contextlib.nullcontext()
            with (
                tc.tile_pool(name="xsl", bufs=4) as xp,
                tc.tile_pool(name="psA", bufs=1, space="PSUM") as pa,
                tc.tile_pool(name="psB", bufs=1, space="PSUM") as pb,
                tc.tile_pool(name="psC", bufs=1, space="PSUM") as pc,
                tc.tile_pool(name="psY", bufs=1, space="PSUM") as py,
                tc.tile_pool(name="psT", bufs=1, space="PSUM") as pt,
                tc.tile_pool(name="epool", bufs=3) as ep,
                tc.tile_pool(name="small", bufs=3) as sp,
                tc.tile_pool(name="ocp", bufs=3) as op_,
                loop_ctx,
            ):
                _ps_ctr = [0]

                def next_ps(shape, name):
                    pool, tag = (
                        (pa, "psA"), (pb, "psB"), (pc, "psC")
                    )[_ps_ctr[0] % 3]
                    _ps_ctr[0] += 1
                    return pool.tile(shape, f32, tag=tag, name=name)

                def load_xq(qb):
                    xq = xp.tile([128, 6, 512], bf16, tag="xq", name=f"xq{qb}")
                    nc.sync.dma_start(
                        out=xq[:], in_=xTqr[:, :, qb * 512:(qb + 1) * 512]
                    )
                    return xq

                def load_xs(nb):
                    xs = xp.tile([128, 6, 512], bf16, tag="xq", name=f"xs{nb}")
                    nc.sync.dma_start(
                        out=xs[:], in_=xTr[:, :, nb * 512:(nb + 1) * 512]
                    )
                    return xs

                def qproj(qb, xq=None):
                    if xq is None:
                        xq = load_xq(qb)
                    for i, (msz, off, QT_t, bq_t) in enumerate(
                        ((128, 0, QT0, bq0), (64, 128, QT1, bq1))
                    ):
                        ps = next_ps([msz, 512], f"qps{qb}_{i}")
                        for k in range(6):
                            nc.tensor.matmul(
                                ps[:],
                                wq_sb[:, k, off:off + msz],
                                xq[:, k, :],
                                start=(k == 0),
                                stop=(k == 5),
                            )
                        nc.vector.tensor_scalar_add(
                            QT_t[:, qb * 512:(qb + 1) * 512], ps[:], bq_t[:]
                        )

                def kproj(nb, xs=None):
                    if xs is None:
                        xs = load_xs(nb)
                    for i, (msz, off, KT_t, bk_t) in enumerate(
                        ((128, 0, KT0, bk0), (64, 128, KT1, bk1))
                    ):
                        ps = next_ps([msz, 512], f"kps{nb}_{i}")
                        for k in range(6):
                            nc.tensor.matmul(
                                ps[:],
                                wk_sb[:, k, off:off + msz],
                                xs[:, k, :],
                                start=(k == 0),
                                stop=(k == 5),
                            )
                        nc.vector.tensor_scalar_add(
                            KT_t[:, nb * 512:(nb + 1) * 512], ps[:], bk_t[:]
                        )

                # V projection: own DMAs -> fills PE gaps under the
                # exp-bound attention phase. Needed only by phase 2.
                def vproj(nb):
                    xv = xp.tile([128, 6, 512], bf16, tag="xv", name=f"xv{nb}", bufs=2)
                    nc.sync.dma_start(
                        out=xv[:], in_=xTr[:, :, nb * 512:(nb + 1) * 512]
                    )
                    for ms in range(4):
                        kt = nb * 4 + ms
                        ps = pt.tile([128, GD], f32, tag="psT", name=f"vps{nb}_{ms}")
                        for k in range(6):
                            nc.tensor.matmul(
                                ps[:],
                                xv[:, k, ms * 128:(ms + 1) * 128],
                                wv_sb[:, k, :],
                                start=(k == 0),
                                stop=(k == 5),
                            )
                        nc.vector.tensor_tensor(
                            Vb[:, :, kt, 0:64],
                            ps[:].rearrange("p (h d) -> p h d", h=3),
                            bvb[:].rearrange("p (h d) -> p h d", h=3),
                            add,
                        )

                # ------------- attention (paired units, h-major) ------------
                # Units are processed in pairs (qb=0 with qb=3, qb=1 with
                # qb=2) so the exact-exp stream (ScalarE, heavy for late qb)
                # and the fast-exp stream (DVE, heavy for early qb) stay
                # concurrently busy. Score tiles go through three rotating
                # 2-bank PSUM pools in uniform 2-key-tile groups.
                e_tiles = {}

                def unit_scores(qb, h, gs):
                    limg = 4 * qb + 4  # first strictly-future 2-kt group
                    if h < 2:
                        KTh = KT0[64 * h:64 * (h + 1), :]
                        QTh = QT0[64 * h:64 * (h + 1), :]
                    else:
                        KTh = KT1[0:64, :]
                        QTh = QT1[0:64, :]
                    if (qb, h) not in e_tiles:
                        e_tiles[(qb, h)] = ep.tile(
                            [128, NK, 512], bf16, tag="E", name=f"E{qb}_{h}"
                        )
                    E = e_tiles[(qb, h)]
                    for g in gs:
                      with tc.high_priority(offset=PRIO_OFF):
                        ps = next_ps([128, 1024], f"sc{qb}_{h}_{g}")
                        for i in range(2):
                            kt = 2 * g + i
                            nc.tensor.matmul(
                                ps[:, i * 512:(i + 1) * 512],
                                KTh[:, kt * 128:(kt + 1) * 128],
                                QTh[:, qb * 512:(qb + 1) * 512],
                                start=True,
                                stop=True,
                            )
                        if g < limg:
                            # numerator-relevant tiles: exact exp
                            nc.scalar.activation(
                                E[:, 2 * g:2 * g + 2, :], ps[:], Exp
                            )
                        else:
                            # denominator-only tiles: fast exp on DVE
                            nc.vector.tensor_scalar(
                                E[:, 2 * g:2 * g + 2, :].bitcast(i16),
                                ps[:],
                                SCH_A,
                                SCH_B,
                                mult,
                                add,
                            )

                def unit_tail(qb, h):
                    E = e_tiles.pop((qb, h))
                    # diagonal-window masked numerators (GpSimd): only the 2
                    # key tiles that can be partially masked for this tile
                    # parity on either shard; earlier diagonal tiles are
                    # fully causal on both shards and later ones fully
                    # masked (mask data is per-core; program is shared).
                    em_all = {}
                    for tl_i in range(4):
                        t = qb * 4 + tl_i
                        k4 = 4 * (t // 2)
                        par = tl_i % 2
                        kcut = k4 + 2 * par
                        tsl = slice(tl_i * 128, (tl_i + 1) * 128)
                        em = sp.tile(
                            [128, 2, 128], bf16, tag="em",
                            name=f"em{t}_{h}", bufs=6,
                        )
                        nc.gpsimd.tensor_tensor(
                            em[:], E[:, kcut:kcut + 2, tsl],
                            mm_sb[:, par, :, :], mult,
                        )
                        em_all[tl_i] = em
                    yp4 = py.tile([128, 4, 65], f32, tag="psY", name=f"yp{qb}_{h}")
                    for tl_i in range(4):
                        t = qb * 4 + tl_i
                        k4 = 4 * (t // 2)
                        kcut = k4 + 2 * (tl_i % 2)
                        tsl = slice(tl_i * 128, (tl_i + 1) * 128)
                        yp = yp4[:, tl_i, :]
                        if kcut == 0:
                            # open the accumulation group across the whole
                            # [0:65] region with a zero contribution
                            nc.tensor.matmul(
                                yp, E[:, 0, tsl], zvb[:],
                                start=True, stop=False,
                            )
                        # causal-on-both-shards tiles: E @ [V|ones]
                        for kt in range(kcut):
                            nc.tensor.matmul(
                                yp,
                                E[:, kt, tsl],
                                Vb[:, h, kt, :],
                                start=(kt == 0),
                                stop=False,
                            )
                        # maybe-masked tiles: masked numerator (64 cols)
                        for j in range(2):
                            kt = kcut + j
                            nc.tensor.matmul(
                                yp4[:, tl_i, 0:64],
                                em_all[tl_i][:, j, :],
                                Vb[:, h, kt, 0:64],
                                start=False,
                                stop=False,
                            )
                        # denominator: full-E column sums for kt >= kcut
                        for kt in range(kcut, NK):
                            nc.tensor.matmul(
                                yp4[:, tl_i, 64:65],
                                E[:, kt, tsl],
                                ones[:],
                                start=False,
                                stop=(kt == NK - 1),
                            )
                    # batched reciprocal + broadcast scale
                    rc4 = sp.tile([128, 4], f32, tag="rc", name=f"rc{qb}_{h}")
                    nc.vector.reciprocal(rc4[:], yp4[:, :, 64].unsqueeze(2))
                    ysb4 = sp.tile(
                        [128, 4, 64], bf16, tag="ysb", name=f"ysb{qb}_{h}"
                    )
                    nc.vector.tensor_tensor(
                        ysb4[:],
                        yp4[:, :, 0:64],
                        rc4[:].unsqueeze(2).broadcast_to([128, 4, 64]),
                        mult,
                    )
                    tp4 = pt.tile([64, 4, 128], bf16, tag="psT", name=f"tp{qb}_{h}")
                    for tl_i in range(4):
                        nc.tensor.transpose(
                            tp4[:, tl_i, :], ysb4[:, tl_i, :], id_sb[:]
                        )
                    if h < 2:
                        dst = yT0[64 * h:64 * (h + 1), qb * 512:(qb + 1) * 512]
                    else:
                        dst = yT1[0:64, qb * 512:(qb + 1) * 512]
                    nc.vector.tensor_copy(dst, tp4[:])

                def outproj_mt(qb, mt, rot=False):
                    if rot:
                        ps = next_ps([128, 512], f"ops{qb}_{mt}")
                    else:
                        ps = pt.tile(
                            [128, 512], f32, tag="psT", name=f"ops{qb}_{mt}"
                        )
                    nc.tensor.matmul(
                        ps[:],
                        wo0[:, mt * 128:(mt + 1) * 128],
                        yT0[:, qb * 512:(qb + 1) * 512],
                        start=True,
                        stop=False,
                    )
                    nc.tensor.matmul(
                        ps[:],
                        wo1[:, mt * 128:(mt + 1) * 128],
                        yT1[:, qb * 512:(qb + 1) * 512],
                        start=False,
                        stop=True,
                    )
                    oc = op_.tile([128, 512], f32, tag="ocp", name=f"oc{qb}_{mt}")
                    if mt % 2 == 0:
                        nc.vector.tensor_copy(oc[:], ps[:])
                    else:
                        nc.scalar.copy(oc[:], ps[:])
                    nc.sync.dma_start(
                        out=d_out[mt * 128:(mt + 1) * 128, qb * 512:(qb + 1) * 512],
                        in_=oc[:],
                    )

                def outproj(qb):
                    for mt in range(6):
                        outproj_mt(qb, mt)

                # --- emission schedule ---
                # The first pair's score groups are emitted interleaved
                # with the K projection in bands of 4 key tiles, so the
                # exp engines start as soon as the first K columns exist
                # (pool FIFO stays producer-before-consumer). All later
                # pairs interleave their groups one-by-one so consecutive
                # pool slots alternate between the ScalarE (exact) and DVE
                # (fast-exp) consumers.
                qproj(0)
                qproj(3)
                # pair 1 scores, interleaved with the K projection bands
                for ki in range(8):
                    kproj(ki)
                    for g in range(2 * ki, 2 * ki + 2):
                        unit_scores(0, 0, [g])
                        unit_scores(3, 0, [g])
                nc.sync.dma_start(out=wv_sb[:], in_=wvr[:])
                nc.sync.dma_start(out=wo0[:], in_=d_wo[0:128, :])
                nc.sync.dma_start(out=wo1[:], in_=d_wo[128:GD, :])
                nc.sync.dma_start(out=bvb[:], in_=d_bvb[:, :])
                nc.sync.dma_start(out=mm_sb[:], in_=mmr[:])
                nc.sync.dma_start(out=id_sb[:], in_=d_id[:, :])
                for nb in range(8):
                    vproj(nb)
                unit_tail(0, 0)
                unit_tail(3, 0)
                for h in (1, 2):
                    for g in range(16):
                        unit_scores(0, h, [g])
                        unit_scores(3, h, [g])
                    unit_tail(0, h)
                    unit_tail(3, h)
                qproj(1)
                qproj(2)
                outproj(0)
                outproj(3)
                for h in range(3):
                    for g in range(16):
                        unit_scores(1, h, [g])
                        unit_scores(2, h, [g])
                    unit_tail(1, h)
                    unit_tail(2, h)
                for mt in range(6):
                    outproj_mt(1, mt, rot=True)
                    outproj_mt(2, mt, rot=True)

    nc.compile()
    return nc


def _get_program(reps=1):
    key = ("nc", reps)
    if key not in _cache:
        _cache[key] = _build_program(reps)
    return _cache[key]


def shard_inputs(x, W_qkv, b_qkv, W_out, b_out):
    """Build the 8 per-core input maps."""
    bf = ml_dtypes.bfloat16
    xT = np.ascontiguousarray(x[0].T.astype(np.float32)).astype(bf)  # [D, S]
    ident = np.eye(128, dtype=np.float32).astype(bf)
    in_maps = []
    per_s = {}
    for s in (0, 1):
        cols = _qcols(s)
        M = _masks(s)
        per_s[s] = (
            np.ascontiguousarray(xT[:, cols]),
            M.astype(bf),
        )
    for c in range(N_CORES):
        hg, s = c // 2, c % 2
        hsl = slice(GD * hg, GD * (hg + 1))
        xTq, M = per_s[s]
        wq = np.ascontiguousarray((W_qkv[0:768][hsl] / 8.0).T.astype(np.float32)).astype(bf)
        wk = np.ascontiguousarray(W_qkv[768:1536][hsl].T.astype(np.float32)).astype(bf)
        wv = np.ascontiguousarray(W_qkv[1536:2304][hsl].T.astype(np.float32)).astype(bf)
        bq = (b_qkv[0:768][hsl] / 8.0).astype(np.float32).reshape(GD, 1)
        bk = b_qkv[768:1536][hsl].astype(np.float32).reshape(GD, 1)
        bv = b_qkv[1536:2304][hsl].astype(np.float32)
        bvb = np.ascontiguousarray(np.broadcast_to(bv[None, :], (128, GD))).astype(bf)
        wo = np.ascontiguousarray(W_out[:, hsl].T.astype(np.float32)).astype(bf)
        in_maps.append(
            {
                "xt_in": xT,
                "xtq_in": xTq,
                "wq_in": wq,
                "wk_in": wk,
                "wv_in": wv,
                "bq_in": bq,
                "bk_in": bk,
                "bvb_in": bvb,
                "wo_in": wo,
                "mm_in": M,
                "id_in": ident,
            }
        )
    return in_maps


def gather_output(results, b_out):
    out = np.zeros((S, D), np.float32)
    for s in (0, 1):
        acc = np.zeros((2048, D), np.float32)
        for hg in range(4):
            c = hg * 2 + s
            acc += results[c]["outt_out"].T.astype(np.float32)
        out[_qcols(s)] = acc + b_out[None, :].astype(np.float32)
    return out.reshape(1, S, D)


def kernel(x, W_qkv, b_qkv, W_out, b_out):
    from concourse.bass_utils import run_bass_kernel_spmd

    x = np.asarray(x)
    W_qkv = np.asarray(W_qkv)
    b_qkv = np.asarray(b_qkv)
    W_out = np.asarray(W_out)
    b_out = np.asarray(b_out)
    nc = _get_program()
    in_maps = shard_inputs(x, W_qkv, b_qkv, W_out, b_out)
    res = run_bass_kernel_spmd(nc, in_maps, list(range(N_CORES)))
    return gather_output(res.results, b_out)
